# revision 33
# baseline (speedup 1.0000x reference)
"""Trainium2 Bass kernel for CapsuleLayer dynamic routing.

Problem: u = einsum('bpe,pjed->bpjd', inp, W[0]) + b, then 3 routing
iterations (softmax over j, weighted sum over p, squash) -> vj [B,J,D].

Shapes: B=16, P=1024, J=32, Dp=D=64.  W is 512MB fp32 -> DMA dominated.

Strategy (8 NeuronCores) -- TENSOR PARALLEL OVER j:
 - Shard J across cores: 4 output capsules per core, ALL 1024 p's and
   all batches local.  The weighted p-sum and the squash are then fully
   core-local; the ONLY collective is the softmax denominator
   Z[b,p] = sum_j exp(b_ipj), a 64KB AllReduce per iteration, whose
   [128,128] layout needs no repacking at all.
 - W packed bf16, columns (d, j4): per pair of p's a [128=(2p x 64e),
   256] panel; 4MiB DMA blocks of 64 pairs.  lhsT[pair] = block-diag
   x so one N=256 matmul computes u for 2 capsules; 4 pairs run in the
   4 PE column groups.
 - u layout [128=(k8,b16), (g'=128, d, j4)] with p = 8*g' + k.
 - Weighted sum via 0/1 Delta matmuls: chunk k of 32 g' accumulates in
   PSUM at partition offset 32k (4 concurrent column groups); the four
   stripe partials are summed AND broadcast to 128 partitions by one
   eye128 matmul (eye128[q,m] = ((q&31)==(m&15))).
 - Squash decoupled: raw s is broadcast, f = scale^2*sqrt(t)/(1+s^2 t)
   is folded into the broadcast vector (vt = s*f) so the agreement
   writes b directly and exp/AR-trigger depend on no extra DVE op.
 - The Z AllReduce ships exp(b) itself (bf16), split in two g'-halves:
   half A triggers mid-agreement, half B hides under the ue = u*exp(b)
   muls; 1/Z is folded into the delta matmul weights (exact, since it
   is per-partition within each g' column block).
 - Final s2 stripes DMA out; host sums stripes + squashes; host
   concatenates the disjoint per-core j-slices.
"""

import numpy as np
import ml_dtypes

import concourse.bass as bass
import concourse.tile as tile
from concourse import bacc, mybir
from concourse.bass_utils import run_bass_kernel_spmd

F32 = mybir.dt.float32
BF16 = mybir.dt.bfloat16
AX = mybir.AxisListType
AF = mybir.ActivationFunctionType

B = 16      # batch
J = 32      # output capsules (total)
D = 64      # output capsule dim
E = 64      # input capsule dim
JD = J * D  # 2048


def build_program(n_cores: int):
    """Per core: J_loc = J/n_cores = 4 capsules, all P=1024 prev capsules."""
    JL = J // n_cores          # 4 local output capsules
    C = D * JL                 # 256 u columns per p
    P = 1024
    NP = P // 2                # 512 pairs
    GP = P // 8                # 128 groups g' (8 p's each)
    RND = GP // 2              # 64 psum rounds (2 g' each)
    NBLK = NP // 16            # 32 DMA blocks: 16 pairs = 1 MiB each

    nc = bacc.Bacc("TRN2", target_bir_lowering=False, debug=False,
                   num_devices=n_cores)

    w_dram = nc.dram_tensor("w", [NBLK, 128, 16 * C], BF16, kind="ExternalInput")
    x_dram = nc.dram_tensor("x", [128, NP * 32], BF16, kind="ExternalInput")
    out_dram = nc.dram_tensor("out", [16, 16 * C], F32, kind="ExternalOutput")

    with tile.TileContext(nc) as tc:
        with (
            tc.tile_pool(name="const", bufs=1) as constp,
            tc.tile_pool(name="wpool", bufs=2) as wpool,
            tc.tile_pool(name="upool", bufs=1) as upool,
            tc.tile_pool(name="work", bufs=2) as work,
            tc.tile_pool(name="small", bufs=1) as small,
            tc.tile_pool(name="pmain", bufs=2, space="PSUM") as pmain,
            tc.tile_pool(name="pacc", bufs=1, space="PSUM") as pacc,
            tc.tile_pool(name="dram", bufs=1, space="DRAM") as dramp,
        ):
            # ---- W streaming (start ASAP) ----
            wdma_engines = [nc.sync, nc.scalar]
            wtiles = {}

            def load_w(blk):
                wt = wpool.tile([128, 16 * C], BF16, tag="w",
                                name=f"wt{blk}", bufs=4)
                wdma_engines[blk % 2].dma_start(wt[:], w_dram[blk])
                return wt

            # ---- static inputs -> SBUF (x chunk 0 first: rounds need it) ----
            x_sb = constp.tile([128, NP * 32], BF16)
            XQ = NP * 8
            nc.sync.dma_start(x_sb[:, :XQ], x_dram[:, :XQ])
            nc.scalar.dma_start(x_sb[:, XQ:2 * XQ], x_dram[:, XQ:2 * XQ])
            for blk in range(3):
                wtiles[blk] = load_w(blk)
            nc.sync.dma_start(x_sb[:, 2 * XQ:3 * XQ], x_dram[:, 2 * XQ:3 * XQ])
            nc.scalar.dma_start(x_sb[:, 3 * XQ:], x_dram[:, 3 * XQ:])
            # 0/1 mask constants built on-device:
            # delta[q, m] = (q % 16 == m);  eye128[q, m] = ((q&31) == (m&15))
            I32 = mybir.dt.int32
            delta_sb = constp.tile([128, 16], BF16)
            qi = small.tile([128, 256], I32, tag="so", name="qi", bufs=2)
            mi = small.tile([128, 256], I32, tag="so", name="mi", bufs=2)
            nc.gpsimd.iota(qi[:, :16], pattern=[[0, 16]], base=0,
                           channel_multiplier=1)
            nc.vector.tensor_scalar(qi[:, :16], qi[:, :16], 15, None,
                                    op0=mybir.AluOpType.bitwise_and)
            nc.gpsimd.iota(mi[:, :16], pattern=[[1, 16]], base=0,
                           channel_multiplier=0)
            nc.vector.tensor_tensor(delta_sb[:], qi[:, :16], mi[:, :16],
                                    op=mybir.AluOpType.is_equal)
            eye_sb = constp.tile([128, 128], BF16)
            nc.gpsimd.iota(qi[:, :128], pattern=[[0, 128]], base=0,
                           channel_multiplier=1)
            nc.vector.tensor_scalar(qi[:, :128], qi[:, :128], 31, None,
                                    op0=mybir.AluOpType.bitwise_and)
            nc.gpsimd.iota(mi[:, :128], pattern=[[1, 128]], base=0,
                           channel_multiplier=0)
            nc.vector.tensor_scalar(mi[:, :128], mi[:, :128], 15, None,
                                    op0=mybir.AluOpType.bitwise_and)
            nc.vector.tensor_tensor(eye_sb[:], qi[:, :128], mi[:, :128],
                                    op=mybir.AluOpType.is_equal)

            u_sb = upool.tile([128, GP * C], BF16)
            # stripe staging buffer: only partitions 32c..32c+16 written;
            # memset once so the eye128 matmul never reads junk.
            s_loc = constp.tile([128, C], BF16)
            nc.vector.memset(s_loc[:], 0.0)
            s_locs = constp.tile([128, 4 * C], BF16)
            nc.vector.memset(s_locs[:], 0.0)

            # ---- collective helpers ----
            n_cc = [0]

            def trigger_ar(cin, shape, dtype=F32):
                i = n_cc[0]
                n_cc[0] += 1
                cout = dramp.tile(shape, dtype, tag=f"cout{i}",
                                  addr_space="Shared" if n_cores > 4 else "Local",
                                  name=f"cc_out{i}")
                nc.gpsimd.collective_compute(
                    "AllReduce", mybir.AluOpType.add,
                    replica_groups=[list(range(n_cores))],
                    ins=[cin.opt()], outs=[cout.opt()],
                )
                return cout

            # ---- phase 1: stream W, matmul u, evict, accumulate s0 ----
            # round r covers g' = 2r, 2r+1 (8 pairs); s0 delta matmuls are
            # col-group tiled by r%4 and issued one round late so the PE
            # never stalls on the DVE eviction.
            ps0 = pacc.tile([128, C], F32, tag="pacc", name="ps0")
            pv0 = pacc.tile([128, C], F32, tag="pv", name="pv0")
            pend = []

            def flush_delta(ps, draining):
                # stripe c accumulates round quarter [16c, 16c+16); its
                # eviction + eye-matmul fold run as soon as it completes,
                # hidden under the ongoing W stream.
                off, r = pend.pop(0)
                c = r // 16
                for gl in range(2):
                    nc.tensor.matmul(
                        ps[32 * c:32 * c + 16, :],
                        delta_sb[:],
                        u_sb[:, off + gl * C: off + (gl + 1) * C],
                        tile_position=(0, 32 * c),
                        start=(r % 16 == 0 and gl == 0),
                        stop=(r % 16 == 15 and gl == 1),
                        skip_group_check=True,
                    )
                if r % 16 == 15:
                    nc.scalar.copy(s_loc[32 * c:32 * c + 16, :],
                                   ps[32 * c:32 * c + 16, :])
                    nc.tensor.matmul(
                        pv0[:, :], eye_sb[32 * c:32 * c + 32, :],
                        s_loc[32 * c:32 * c + 32, :],
                        tile_position=(32 * c, 0),
                        start=(c == 0), stop=(c == 3),
                        skip_group_check=True,
                    )

            for r in range(RND):
                blk = r // 2
                if r % 2 == 0 and blk + 3 < NBLK and (blk + 3) not in wtiles:
                    wtiles[blk + 3] = load_w(blk + 3)
                wt = wtiles[blk]
                pm = pmain.tile([128, 2 * C], F32, tag="pmain", name=f"pm{r}")
                for cg in range(4):
                    for gl in range(2):
                        pr = (2 * r + gl) * 4 + cg          # global pair index
                        lhsT = x_sb[:, pr * 32:(pr + 1) * 32]
                        pl = pr - 16 * blk                  # pair within block
                        nc.tensor.matmul(
                            pm[32 * cg:32 * cg + 32, gl * C:(gl + 1) * C],
                            lhsT,
                            wt[:, pl * C:(pl + 1) * C],
                            tile_position=(0, 32 * cg),
                        )
                if r % 2 == 1:
                    wtiles.pop(blk, None)
                off = r * 2 * C
                # drain the tail on ACT: the DVE queue is backlogged with
                # earlier evictions, and the last eviction gates the final
                # s0 delta -> stripe -> eye -> v_sb chain before iter 1
                ev_eng = nc.scalar if r >= RND - 3 else nc.vector
                if ev_eng is nc.scalar:
                    nc.scalar.copy(u_sb[:, off:off + 2 * C], pm[:])
                else:
                    nc.vector.tensor_copy(u_sb[:, off:off + 2 * C], pm[:])
                pend.append((off, r))
                if len(pend) > 1:
                    flush_delta(ps0, False)
                if r == RND - 1:
                    while pend:
                        flush_delta(ps0, True)

            # ---- fold stripes + broadcast raw s to 128 partitions ----
            v_sb = constp.tile([128, C], BF16)

            # ---- squash factor f = scale^2*sqrt(t) / (1 + scale^2*t) ----
            def f_chain(scale, it):
                s2v = small.tile([128, C], BF16, tag="s2v", name=f"s2v{it}")
                nc.vector.tensor_mul(s2v[:], v_sb[:], v_sb[:])
                n = C // 2
                while n >= JL * 2:
                    nc.vector.tensor_add(s2v[:, :n], s2v[:, :n], s2v[:, n:2 * n])
                    n //= 2
                t = small.tile([128, JL], F32, tag="t", name=f"t{it}")
                nc.vector.tensor_add(t[:], s2v[:, :JL], s2v[:, JL:2 * JL])
                st = small.tile([128, JL], F32, tag="st", name=f"st{it}")
                nc.scalar.sqrt(st[:], t[:])
                den = small.tile([128, JL], F32, tag="den", name=f"den{it}")
                nc.vector.tensor_scalar(den[:], t[:], scale * scale, 1.0,
                                        op0=mybir.AluOpType.mult,
                                        op1=mybir.AluOpType.add)
                rec = small.tile([128, JL], F32, tag="rec", name=f"rec{it}")
                nc.vector.reciprocal(rec[:], den[:])
                f = small.tile([128, JL], F32, tag="f", name=f"f{it}")
                nc.vector.scalar_tensor_tensor(f[:], st[:], scale * scale, rec[:],
                                               op0=mybir.AluOpType.mult,
                                               op1=mybir.AluOpType.mult)
                return f

            nc.scalar.copy(v_sb[:], pv0[:, :])

            # ---- routing iterations (all elementwise work on DVE) ----
            braw = constp.tile([128, GP * JL], F32)
            bij = constp.tile([128, GP * JL], F32)
            NCH = 4
            GC = GP // NCH           # 32 g' per chunk

            def agree_chunk(k, b_h, vt):
                ch = GC * C
                u_ch = u_sb[:, k * ch:(k + 1) * ch]
                tmp = work.tile([128, ch], BF16, tag="tmpv", name="tmp", bufs=4)
                nc.vector.tensor_mul(
                    tmp[:].rearrange("p (g q) -> p g q", g=GC),
                    u_ch.rearrange("p (g q) -> p g q", g=GC),
                    vt[:].unsqueeze(1).broadcast_to([128, GC, C]),
                )
                r3 = tmp[:].rearrange("p (g q) -> p g q", g=GC)
                dl = D
                while dl > 2:
                    half = dl // 2 * JL
                    nc.vector.tensor_add(
                        r3[:, :, 0:half], r3[:, :, 0:half], r3[:, :, half:2 * half])
                    dl //= 2
                nc.vector.tensor_add(
                    b_h.rearrange("p (g j) -> p g j", g=GC),
                    r3[:, :, 0:JL], r3[:, :, JL:2 * JL])

            for it in (1, 2):
                scale = (1.0 / J) if it == 1 else 1.0
                # fold the squash factor into the broadcast vector once:
                # the agreement then writes b = f * <s, u> directly and no
                # DVE op sits between the last tree and the AR trigger.
                f = f_chain(scale, it)
                vt = small.tile([128, C], BF16, tag="vt", name=f"vt{it}")
                nc.vector.tensor_mul(
                    vt[:].rearrange("p (d j) -> p d j", d=D),
                    v_sb[:].rearrange("p (d j) -> p d j", d=D),
                    f[:].unsqueeze(1).broadcast_to([128, D, JL]))
                tgt = bij if it == 1 else braw
                eh16 = small.tile([128, GP * JL], BF16, tag="eh", name=f"eh{it}")
                HC = GP * JL // 2
                couts = []
                # AllReduce exp(b) itself (bf16, split in two g'-halves): the
                # elementwise sum across cores plus a local 4-slot j reduce
                # after the AR is exactly Z.  Half A triggers mid-agreement
                # and hides completely; half B hides under the ue muls.
                for k in range(NCH):
                    sl = slice(k * GC * JL, (k + 1) * GC * JL)
                    agree_chunk(k, tgt[:, sl], vt)
                    if k % 2 == 1:
                        h = k // 2
                        hs = slice(h * HC, (h + 1) * HC)
                        if it == 2:
                            # on GpSimd: a DVE add here has only cross-engine
                            # successors and gets scheduled after the ue muls,
                            # delaying the AR trigger by ~20us
                            nc.gpsimd.tensor_add(bij[:, hs], bij[:, hs],
                                                 braw[:, hs])
                        nc.scalar.activation(eh16[:, hs], bij[:, hs], AF.Exp)
                        cin = dramp.tile([128, HC], BF16, tag=f"cin{it}{h}",
                                         name=f"cc_z{it}{h}")
                        nc.sync.dma_start(cin[:], eh16[:, hs])
                        couts.append(trigger_ar(cin, [128, HC], BF16))
                # ue = u * e needs no Z: it runs while the AllReduce flies.
                # 1/Z is folded into the delta matmul weights instead
                # (w[q, b] = delta[q, b] * re[q, g'] -- per-partition exact).
                ues = []
                for k in range(NCH):
                    ch = GC * C
                    u_ch = u_sb[:, k * ch:(k + 1) * ch]
                    u4 = u_ch.rearrange("p (g d j) -> p g d j", g=GC, d=D)
                    ebc = (eh16[:, k * GC * JL:(k + 1) * GC * JL]
                           .rearrange("p (g j) -> p g j", g=GC)
                           .unsqueeze(2).broadcast_to([128, GC, D, JL]))
                    if it == 2:
                        # u is dead after this iteration: multiply in place,
                        # so no work-buffer rotation gates the overlap
                        nc.vector.tensor_mul(u4, u4, ebc)
                        ues.append(u_ch)
                    else:
                        ue = work.tile([128, ch], BF16, tag="tmpv", name="ue",
                                       bufs=4)
                        nc.vector.tensor_mul(
                            ue[:].rearrange("p (g d j) -> p g d j", g=GC, d=D),
                            u4, ebc)
                        ues.append(ue)
                if it == 1:
                    pv = pacc.tile([128, C], F32, tag="pv", name="pv1")
                res = {}
                for k in range(NCH):
                    h = k // 2
                    if h not in res:
                        # fetch + j-reduce + recip per AR half; these DVE ops
                        # have no DVE successors (wsc is on GpSimd) so the
                        # scheduler won't hoist them ahead of the ue muls.
                        zf = small.tile([128, HC], BF16, tag="so",
                                        name=f"zf{it}{h}", bufs=2)
                        nc.sync.dma_start(zf[:], couts[h][:])
                        zs = small.tile([128, GP // 2], F32, tag=f"zs{h}",
                                        name=f"zs{it}{h}")
                        nc.vector.reduce_sum(
                            zs[:], zf[:].rearrange("p (g j) -> p g j",
                                                   g=GP // 2), axis=AX.X)
                        re = small.tile([128, GP // 2], F32, tag=f"re{h}",
                                        name=f"re{it}{h}")
                        nc.vector.reciprocal(re[:], zs[:])
                        res[h] = re
                    re = res[h]
                    ue = ues[k]
                    if it != 2:
                        ue = ue[:]
                    wsc = small.tile([128, GC * 16], BF16, tag="wsc",
                                     name=f"wsc{k}", bufs=2)
                    nc.gpsimd.tensor_mul(
                        wsc[:].rearrange("p (g m) -> p g m", g=GC),
                        re[:, (k % 2) * GC:(k % 2 + 1) * GC]
                            .unsqueeze(2).broadcast_to([128, GC, 16]),
                        delta_sb[:].unsqueeze(1).broadcast_to([128, GC, 16]),
                    )
                    # chunk-private accumulator: its 4 stripes complete with
                    # the chunk, so fold/out overlap the next chunk's work
                    ps = pacc.tile([128, C], F32, tag=f"psk{k}", name=f"ps{it}_{k}", bufs=1)
                    for gg in range(GC):
                        c = gg % 4
                        nc.tensor.matmul(
                            ps[32 * c:32 * c + 16, :],
                            wsc[:, gg * 16:(gg + 1) * 16],
                            ue[:, gg * C:(gg + 1) * C],
                            tile_position=(0, 32 * c),
                            start=(gg == c),
                            stop=(gg == GC - 4 + c),
                            skip_group_check=True,
                        )
                    if it == 1:
                        for c in range(4):
                            if c % 2 == 0:
                                nc.scalar.copy(
                                    s_locs[32 * c:32 * c + 16, k * C:(k + 1) * C],
                                    ps[32 * c:32 * c + 16, :])
                            else:
                                nc.vector.tensor_copy(
                                    s_locs[32 * c:32 * c + 16, k * C:(k + 1) * C],
                                    ps[32 * c:32 * c + 16, :])
                        nc.tensor.matmul(
                            pv[:, :], eye_sb[:],
                            s_locs[:, k * C:(k + 1) * C],
                            start=(k == 0), stop=(k == NCH - 1),
                            skip_group_check=True,
                        )
                    else:
                        s_out = small.tile([128, C], F32, tag="so",
                                           name=f"s_out{k}", bufs=2)
                        for c in range(4):
                            if c % 2 == 0:
                                nc.scalar.copy(s_out[32 * c:32 * c + 16, :],
                                               ps[32 * c:32 * c + 16, :])
                            else:
                                nc.vector.tensor_copy(
                                    s_out[32 * c:32 * c + 16, :],
                                    ps[32 * c:32 * c + 16, :])
                            wdma_engines[c % 2].dma_start(
                                out_dram[:, (4 * k + c) * C:(4 * k + c + 1) * C],
                                s_out[32 * c:32 * c + 16, :])
                if it == 1:
                    nc.scalar.copy(v_sb[:], pv[:, :])

    nc.compile()
    return nc


def pack_inputs(inp, W, b, n_cores: int):
    """Host-side packing -> per-core in_maps. W sharded by j, cols (d, j4)."""
    P = inp.shape[1]
    JL = J // n_cores
    C = D * JL
    NP = P // 2
    NBLK = NP // 16

    bf = ml_dtypes.bfloat16
    if b is not None and np.any(b):
        raise NotImplementedError("nonzero bias b is not supported")
    # W[0]: [P, J, E, D] -> per core [P, E, (D, JL)] -> pairs [NP, 128, C]
    Wc = W[0].reshape(P, n_cores, JL, E, D)          # [P, c, jl, E, D]
    Wt = np.ascontiguousarray(Wc.transpose(1, 0, 3, 4, 2))  # [c, P, E, D, jl]
    Wp = Wt.reshape(n_cores, NP, 2 * E, C)
    Wb = Wp.reshape(n_cores, NBLK, 16, 2 * E, C).transpose(0, 1, 3, 2, 4)
    w_dev = np.ascontiguousarray(Wb).reshape(n_cores, NBLK, 128, 16 * C).astype(bf)

    # x: [B, P, E] -> block diag lhsT [128, NP*32], same on every core
    inpT = inp.transpose(1, 2, 0)          # [P, E, B]
    arr = inpT.reshape(NP, 2, E, B)
    x_dev = np.zeros((2, E, NP, 2, 16), np.float32)
    x_dev[0, :, :, 0, :] = arr[:, 0].transpose(1, 0, 2)
    x_dev[1, :, :, 1, :] = arr[:, 1].transpose(1, 0, 2)
    x_dev = x_dev.reshape(128, NP * 32).astype(bf)

    in_maps = []
    for c in range(n_cores):
        in_maps.append({"w": w_dev[c], "x": x_dev})
    return in_maps


def squash_np(x):
    s2 = np.sum(x * x, axis=-1, keepdims=True)
    return x * (s2 / (1.0 + s2)) / np.sqrt(s2)


def unshard(results):
    """Per-core 'out' [16, 4*C]: 4 stripe partials of s2 [16, (D, JL)].
    Sum stripes, squash, concatenate the disjoint j slices."""
    n_cores = len(results)
    JL = J // n_cores
    C = D * JL
    v = np.zeros((B, J, D), np.float32)
    for ci, r in enumerate(results):
        s = r["out"].astype(np.float64).reshape(16, 16, C).sum(axis=1)
        vj = squash_np(s.reshape(B, D, JL).transpose(0, 2, 1))
        v[:, ci * JL:(ci + 1) * JL, :] = vj
    return v


_CACHE = {}


def kernel(inp: np.ndarray, W: np.ndarray, b: np.ndarray) -> np.ndarray:
    n_cores = 8
    inp = np.asarray(inp, dtype=np.float32)
    W = np.asarray(W, dtype=np.float32)
    b = np.asarray(b, dtype=np.float32)

    if n_cores not in _CACHE:
        _CACHE[n_cores] = build_program(n_cores)
    nc = _CACHE[n_cores]

    in_maps = pack_inputs(inp, W, b, n_cores)
    res = run_bass_kernel_spmd(nc, in_maps, core_ids=list(range(n_cores)))
    return unshard(res.results)


# revision 34
# speedup vs baseline: 1.0871x; 1.0871x over previous
"""Trainium2 Bass kernel for CapsuleLayer dynamic routing.

Problem: u = einsum('bpe,pjed->bpjd', inp, W[0]) + b, then 3 routing
iterations (softmax over j, weighted sum over p, squash) -> vj [B,J,D].

Shapes: B=16, P=1024, J=32, Dp=D=64.  W is 512MB fp32 -> DMA dominated.

Strategy (8 NeuronCores) -- TENSOR PARALLEL OVER j:
 - Shard J across cores: 4 output capsules per core, ALL 1024 p's and
   all batches local.  The weighted p-sum and the squash are then fully
   core-local; the ONLY collective is the softmax denominator
   Z[b,p] = sum_j exp(b_ipj), a 64KB AllReduce per iteration, whose
   [128,128] layout needs no repacking at all.
 - W packed bf16, columns (d, j4): per pair of p's a [128=(2p x 64e),
   256] panel; 4MiB DMA blocks of 64 pairs.  lhsT[pair] = block-diag
   x so one N=256 matmul computes u for 2 capsules; 4 pairs run in the
   4 PE column groups.
 - u layout [128=(k8,b16), (g'=128, d, j4)] with p = 8*g' + k.
 - Weighted sum via 0/1 Delta matmuls: chunk k of 32 g' accumulates in
   PSUM at partition offset 32k (4 concurrent column groups); the four
   stripe partials are summed AND broadcast to 128 partitions by one
   eye128 matmul (eye128[q,m] = ((q&31)==(m&15))).
 - Squash decoupled: raw s is broadcast, f = scale^2*sqrt(t)/(1+s^2 t)
   is folded into the broadcast vector (vt = s*f) so the agreement
   writes b directly and exp/AR-trigger depend on no extra DVE op.
 - The Z AllReduce ships exp(b) itself (bf16), split in two g'-halves:
   half A triggers mid-agreement, half B hides under the ue = u*exp(b)
   muls; 1/Z is folded into the delta matmul weights (exact, since it
   is per-partition within each g' column block).
 - Final s2 stripes DMA out; host sums stripes + squashes; host
   concatenates the disjoint per-core j-slices.
"""

import numpy as np
import ml_dtypes

import concourse.bass as bass
import concourse.tile as tile
from concourse import bacc, mybir
from concourse.bass_utils import run_bass_kernel_spmd

F32 = mybir.dt.float32
BF16 = mybir.dt.bfloat16
AX = mybir.AxisListType
AF = mybir.ActivationFunctionType

B = 16      # batch
J = 32      # output capsules (total)
D = 64      # output capsule dim
E = 64      # input capsule dim
JD = J * D  # 2048


def build_program(n_cores: int):
    """Per core: J_loc = J/n_cores = 4 capsules, all P=1024 prev capsules."""
    JL = J // n_cores          # 4 local output capsules
    C = D * JL                 # 256 u columns per p
    P = 1024
    NP = P // 2                # 512 pairs
    GP = P // 8                # 128 groups g' (8 p's each)
    RND = GP // 2              # 64 psum rounds (2 g' each)
    NBLK = NP // 16            # 32 DMA blocks: 16 pairs = 1 MiB each

    nc = bacc.Bacc("TRN2", target_bir_lowering=False, debug=False,
                   num_devices=n_cores)

    w_dram = nc.dram_tensor("w", [NBLK, 128, 16 * C], BF16, kind="ExternalInput")
    x_dram = nc.dram_tensor("x", [128, NP * 32], BF16, kind="ExternalInput")
    out_dram = nc.dram_tensor("out", [16, 16 * C], F32, kind="ExternalOutput")

    with tile.TileContext(nc) as tc:
        with (
            tc.tile_pool(name="const", bufs=1) as constp,
            tc.tile_pool(name="wpool", bufs=2) as wpool,
            tc.tile_pool(name="upool", bufs=1) as upool,
            tc.tile_pool(name="work", bufs=2) as work,
            tc.tile_pool(name="small", bufs=1) as small,
            tc.tile_pool(name="pmain", bufs=2, space="PSUM") as pmain,
            tc.tile_pool(name="pacc", bufs=1, space="PSUM") as pacc,
            tc.tile_pool(name="dram", bufs=1, space="DRAM") as dramp,
        ):
            # ---- W streaming (start ASAP) ----
            wdma_engines = [nc.sync, nc.scalar]
            wtiles = {}

            def load_w(blk):
                wt = wpool.tile([128, 16 * C], BF16, tag="w",
                                name=f"wt{blk}", bufs=4)
                wdma_engines[blk % 2].dma_start(wt[:], w_dram[blk])
                return wt

            # ---- static inputs -> SBUF (x chunk 0 first: rounds need it) ----
            x_sb = constp.tile([128, NP * 32], BF16)
            XQ = NP * 8
            nc.sync.dma_start(x_sb[:, :XQ], x_dram[:, :XQ])
            nc.scalar.dma_start(x_sb[:, XQ:2 * XQ], x_dram[:, XQ:2 * XQ])
            for blk in range(3):
                wtiles[blk] = load_w(blk)
            nc.sync.dma_start(x_sb[:, 2 * XQ:3 * XQ], x_dram[:, 2 * XQ:3 * XQ])
            nc.scalar.dma_start(x_sb[:, 3 * XQ:], x_dram[:, 3 * XQ:])
            # 0/1 mask constants built on-device:
            # delta[q, m] = (q % 16 == m);  eye128[q, m] = ((q&31) == (m&15))
            I32 = mybir.dt.int32
            delta_sb = constp.tile([128, 16], BF16)
            qi = small.tile([128, 256], I32, tag="so", name="qi", bufs=2)
            mi = small.tile([128, 256], I32, tag="so", name="mi", bufs=2)
            nc.gpsimd.iota(qi[:, :16], pattern=[[0, 16]], base=0,
                           channel_multiplier=1)
            nc.vector.tensor_scalar(qi[:, :16], qi[:, :16], 15, None,
                                    op0=mybir.AluOpType.bitwise_and)
            nc.gpsimd.iota(mi[:, :16], pattern=[[1, 16]], base=0,
                           channel_multiplier=0)
            nc.vector.tensor_tensor(delta_sb[:], qi[:, :16], mi[:, :16],
                                    op=mybir.AluOpType.is_equal)
            eye_sb = constp.tile([128, 128], BF16)
            nc.gpsimd.iota(qi[:, :128], pattern=[[0, 128]], base=0,
                           channel_multiplier=1)
            nc.vector.tensor_scalar(qi[:, :128], qi[:, :128], 31, None,
                                    op0=mybir.AluOpType.bitwise_and)
            nc.gpsimd.iota(mi[:, :128], pattern=[[1, 128]], base=0,
                           channel_multiplier=0)
            nc.vector.tensor_scalar(mi[:, :128], mi[:, :128], 15, None,
                                    op0=mybir.AluOpType.bitwise_and)
            nc.vector.tensor_tensor(eye_sb[:], qi[:, :128], mi[:, :128],
                                    op=mybir.AluOpType.is_equal)

            u_sb = upool.tile([128, GP * C], BF16)
            # stripe staging buffer: only partitions 32c..32c+16 written;
            # memset once so the eye128 matmul never reads junk.
            s_loc = constp.tile([128, C], BF16)
            nc.vector.memset(s_loc[:], 0.0)
            s_locs = constp.tile([128, 4 * C], BF16)
            nc.vector.memset(s_locs[:], 0.0)

            # ---- collective helpers ----
            n_cc = [0]

            def trigger_ar(cin, shape, dtype=F32):
                i = n_cc[0]
                n_cc[0] += 1
                cout = dramp.tile(shape, dtype, tag=f"cout{i}",
                                  addr_space="Shared" if n_cores > 4 else "Local",
                                  name=f"cc_out{i}")
                nc.gpsimd.collective_compute(
                    "AllReduce", mybir.AluOpType.add,
                    replica_groups=[list(range(n_cores))],
                    ins=[cin.opt()], outs=[cout.opt()],
                )
                return cout

            # ---- phase 1: stream W, matmul u, evict, accumulate s0 ----
            # round r covers g' = 2r, 2r+1 (8 pairs); s0 delta matmuls are
            # col-group tiled by r%4 and issued one round late so the PE
            # never stalls on the DVE eviction.
            ps0 = pacc.tile([128, C], F32, tag="pacc", name="ps0")
            pv0 = pacc.tile([128, C], F32, tag="pv", name="pv0")
            pend = []

            def flush_delta(ps, draining):
                # stripe c accumulates round quarter [16c, 16c+16); its
                # eviction + eye-matmul fold run as soon as it completes,
                # hidden under the ongoing W stream.
                off, r = pend.pop(0)
                c = r // 16
                for gl in range(2):
                    nc.tensor.matmul(
                        ps[32 * c:32 * c + 16, :],
                        delta_sb[:],
                        u_sb[:, off + gl * C: off + (gl + 1) * C],
                        tile_position=(0, 32 * c),
                        start=(r % 16 == 0 and gl == 0),
                        stop=(r % 16 == 15 and gl == 1),
                        skip_group_check=True,
                    )
                if r % 16 == 15:
                    nc.scalar.copy(s_loc[32 * c:32 * c + 16, :],
                                   ps[32 * c:32 * c + 16, :])
                    nc.tensor.matmul(
                        pv0[:, :], eye_sb[32 * c:32 * c + 32, :],
                        s_loc[32 * c:32 * c + 32, :],
                        tile_position=(32 * c, 0),
                        start=(c == 0), stop=(c == 3),
                        skip_group_check=True,
                    )

            for r in range(RND):
                blk = r // 2
                if r % 2 == 0 and blk + 3 < NBLK and (blk + 3) not in wtiles:
                    wtiles[blk + 3] = load_w(blk + 3)
                wt = wtiles[blk]
                pm = pmain.tile([128, 2 * C], F32, tag="pmain", name=f"pm{r}")
                for cg in range(4):
                    for gl in range(2):
                        pr = (2 * r + gl) * 4 + cg          # global pair index
                        lhsT = x_sb[:, pr * 32:(pr + 1) * 32]
                        pl = pr - 16 * blk                  # pair within block
                        nc.tensor.matmul(
                            pm[32 * cg:32 * cg + 32, gl * C:(gl + 1) * C],
                            lhsT,
                            wt[:, pl * C:(pl + 1) * C],
                            tile_position=(0, 32 * cg),
                        )
                if r % 2 == 1:
                    wtiles.pop(blk, None)
                off = r * 2 * C
                nc.vector.tensor_copy(u_sb[:, off:off + 2 * C], pm[:])
                pend.append((off, r))
                if len(pend) > 1:
                    flush_delta(ps0, False)
                if r == RND - 1:
                    while pend:
                        flush_delta(ps0, True)

            # ---- fold stripes + broadcast raw s to 128 partitions ----
            v_sb = constp.tile([128, C], BF16)

            # ---- squash factor f = scale^2*sqrt(t) / (1 + scale^2*t) ----
            def f_chain(scale, it):
                s2v = small.tile([128, C], BF16, tag="s2v", name=f"s2v{it}")
                nc.vector.tensor_mul(s2v[:], v_sb[:], v_sb[:])
                n = C // 2
                while n >= JL * 2:
                    nc.vector.tensor_add(s2v[:, :n], s2v[:, :n], s2v[:, n:2 * n])
                    n //= 2
                t = small.tile([128, JL], F32, tag="t", name=f"t{it}")
                nc.vector.tensor_add(t[:], s2v[:, :JL], s2v[:, JL:2 * JL])
                st = small.tile([128, JL], F32, tag="st", name=f"st{it}")
                nc.scalar.sqrt(st[:], t[:])
                den = small.tile([128, JL], F32, tag="den", name=f"den{it}")
                nc.vector.tensor_scalar(den[:], t[:], scale * scale, 1.0,
                                        op0=mybir.AluOpType.mult,
                                        op1=mybir.AluOpType.add)
                rec = small.tile([128, JL], F32, tag="rec", name=f"rec{it}")
                nc.vector.reciprocal(rec[:], den[:])
                f = small.tile([128, JL], F32, tag="f", name=f"f{it}")
                nc.vector.scalar_tensor_tensor(f[:], st[:], scale * scale, rec[:],
                                               op0=mybir.AluOpType.mult,
                                               op1=mybir.AluOpType.mult)
                return f

            nc.scalar.copy(v_sb[:], pv0[:, :])

            # ---- routing iterations (all elementwise work on DVE) ----
            braw = constp.tile([128, GP * JL], F32)
            bij = constp.tile([128, GP * JL], F32)
            NCH = 4
            GC = GP // NCH           # 32 g' per chunk

            def agree_chunk(k, b_h, vt):
                ch = GC * C
                u_ch = u_sb[:, k * ch:(k + 1) * ch]
                tmp = work.tile([128, ch], BF16, tag="tmpv", name="tmp", bufs=4)
                nc.vector.tensor_mul(
                    tmp[:].rearrange("p (g q) -> p g q", g=GC),
                    u_ch.rearrange("p (g q) -> p g q", g=GC),
                    vt[:].unsqueeze(1).broadcast_to([128, GC, C]),
                )
                r3 = tmp[:].rearrange("p (g q) -> p g q", g=GC)
                dl = D
                while dl > 2:
                    half = dl // 2 * JL
                    nc.vector.tensor_add(
                        r3[:, :, 0:half], r3[:, :, 0:half], r3[:, :, half:2 * half])
                    dl //= 2
                nc.vector.tensor_add(
                    b_h.rearrange("p (g j) -> p g j", g=GC),
                    r3[:, :, 0:JL], r3[:, :, JL:2 * JL])

            for it in (1, 2):
                scale = (1.0 / J) if it == 1 else 1.0
                # fold the squash factor into the broadcast vector once:
                # the agreement then writes b = f * <s, u> directly and no
                # DVE op sits between the last tree and the AR trigger.
                f = f_chain(scale, it)
                vt = small.tile([128, C], BF16, tag="vt", name=f"vt{it}")
                nc.vector.tensor_mul(
                    vt[:].rearrange("p (d j) -> p d j", d=D),
                    v_sb[:].rearrange("p (d j) -> p d j", d=D),
                    f[:].unsqueeze(1).broadcast_to([128, D, JL]))
                tgt = bij if it == 1 else braw
                eh16 = small.tile([128, GP * JL], BF16, tag="eh", name=f"eh{it}")
                HC = GP * JL // 2
                couts = []
                # AllReduce exp(b) itself (bf16, split in two g'-halves): the
                # elementwise sum across cores plus a local 4-slot j reduce
                # after the AR is exactly Z.  Half A triggers mid-agreement
                # and hides completely; half B hides under the ue muls.
                for k in range(NCH):
                    sl = slice(k * GC * JL, (k + 1) * GC * JL)
                    agree_chunk(k, tgt[:, sl], vt)
                    if k % 2 == 1:
                        h = k // 2
                        hs = slice(h * HC, (h + 1) * HC)
                        if it == 2:
                            # on GpSimd: a DVE add here has only cross-engine
                            # successors and gets scheduled after the ue muls,
                            # delaying the AR trigger by ~20us
                            nc.gpsimd.tensor_add(bij[:, hs], bij[:, hs],
                                                 braw[:, hs])
                        nc.scalar.activation(eh16[:, hs], bij[:, hs], AF.Exp)
                        cin = dramp.tile([128, HC], BF16, tag=f"cin{it}{h}",
                                         name=f"cc_z{it}{h}")
                        nc.sync.dma_start(cin[:], eh16[:, hs])
                        couts.append(trigger_ar(cin, [128, HC], BF16))
                # ue = u * e needs no Z: it runs while the AllReduce flies.
                # 1/Z is folded into the delta matmul weights instead
                # (w[q, b] = delta[q, b] * re[q, g'] -- per-partition exact).
                ues = []
                for k in range(NCH):
                    ch = GC * C
                    u_ch = u_sb[:, k * ch:(k + 1) * ch]
                    u4 = u_ch.rearrange("p (g d j) -> p g d j", g=GC, d=D)
                    ebc = (eh16[:, k * GC * JL:(k + 1) * GC * JL]
                           .rearrange("p (g j) -> p g j", g=GC)
                           .unsqueeze(2).broadcast_to([128, GC, D, JL]))
                    if it == 2:
                        # u is dead after this iteration: multiply in place,
                        # so no work-buffer rotation gates the overlap
                        nc.vector.tensor_mul(u4, u4, ebc)
                        ues.append(u_ch)
                    else:
                        ue = work.tile([128, ch], BF16, tag="tmpv", name="ue",
                                       bufs=4)
                        nc.vector.tensor_mul(
                            ue[:].rearrange("p (g d j) -> p g d j", g=GC, d=D),
                            u4, ebc)
                        ues.append(ue)
                if it == 1:
                    pv = pacc.tile([128, C], F32, tag="pv", name="pv1")
                res = {}
                for k in range(NCH):
                    h = k // 2
                    if h not in res:
                        # fetch + j-reduce + recip per AR half; these DVE ops
                        # have no DVE successors (wsc is on GpSimd) so the
                        # scheduler won't hoist them ahead of the ue muls.
                        zf = small.tile([128, HC], BF16, tag="so",
                                        name=f"zf{it}{h}", bufs=2)
                        nc.sync.dma_start(zf[:], couts[h][:])
                        zs = small.tile([128, GP // 2], F32, tag=f"zs{h}",
                                        name=f"zs{it}{h}")
                        nc.vector.reduce_sum(
                            zs[:], zf[:].rearrange("p (g j) -> p g j",
                                                   g=GP // 2), axis=AX.X)
                        re = small.tile([128, GP // 2], F32, tag=f"re{h}",
                                        name=f"re{it}{h}")
                        nc.vector.reciprocal(re[:], zs[:])
                        res[h] = re
                    re = res[h]
                    ue = ues[k]
                    if it != 2:
                        ue = ue[:]
                    wsc = small.tile([128, GC * 16], BF16, tag="wsc",
                                     name=f"wsc{k}", bufs=2)
                    nc.gpsimd.tensor_mul(
                        wsc[:].rearrange("p (g m) -> p g m", g=GC),
                        re[:, (k % 2) * GC:(k % 2 + 1) * GC]
                            .unsqueeze(2).broadcast_to([128, GC, 16]),
                        delta_sb[:].unsqueeze(1).broadcast_to([128, GC, 16]),
                    )
                    # chunk-private accumulator: its 4 stripes complete with
                    # the chunk, so fold/out overlap the next chunk's work
                    ps = pacc.tile([128, C], F32, tag=f"psk{k}", name=f"ps{it}_{k}", bufs=1)
                    for gg in range(GC):
                        c = gg % 4
                        nc.tensor.matmul(
                            ps[32 * c:32 * c + 16, :],
                            wsc[:, gg * 16:(gg + 1) * 16],
                            ue[:, gg * C:(gg + 1) * C],
                            tile_position=(0, 32 * c),
                            start=(gg == c),
                            stop=(gg == GC - 4 + c),
                            skip_group_check=True,
                        )
                    if it == 1:
                        for c in range(4):
                            if c % 2 == 0:
                                nc.scalar.copy(
                                    s_locs[32 * c:32 * c + 16, k * C:(k + 1) * C],
                                    ps[32 * c:32 * c + 16, :])
                            else:
                                nc.vector.tensor_copy(
                                    s_locs[32 * c:32 * c + 16, k * C:(k + 1) * C],
                                    ps[32 * c:32 * c + 16, :])
                        nc.tensor.matmul(
                            pv[:, :], eye_sb[:],
                            s_locs[:, k * C:(k + 1) * C],
                            start=(k == 0), stop=(k == NCH - 1),
                            skip_group_check=True,
                        )
                    else:
                        s_out = small.tile([128, C], F32, tag="so",
                                           name=f"s_out{k}", bufs=2)
                        for c in range(4):
                            if c % 2 == 0:
                                nc.scalar.copy(s_out[32 * c:32 * c + 16, :],
                                               ps[32 * c:32 * c + 16, :])
                            else:
                                nc.vector.tensor_copy(
                                    s_out[32 * c:32 * c + 16, :],
                                    ps[32 * c:32 * c + 16, :])
                            wdma_engines[c % 2].dma_start(
                                out_dram[:, (4 * k + c) * C:(4 * k + c + 1) * C],
                                s_out[32 * c:32 * c + 16, :])
                if it == 1:
                    nc.scalar.copy(v_sb[:], pv[:, :])

    nc.compile()
    return nc


def pack_inputs(inp, W, b, n_cores: int):
    """Host-side packing -> per-core in_maps. W sharded by j, cols (d, j4)."""
    P = inp.shape[1]
    JL = J // n_cores
    C = D * JL
    NP = P // 2
    NBLK = NP // 16

    bf = ml_dtypes.bfloat16
    if b is not None and np.any(b):
        raise NotImplementedError("nonzero bias b is not supported")
    # W[0]: [P, J, E, D] -> per core [P, E, (D, JL)] -> pairs [NP, 128, C]
    Wc = W[0].reshape(P, n_cores, JL, E, D)          # [P, c, jl, E, D]
    Wt = np.ascontiguousarray(Wc.transpose(1, 0, 3, 4, 2))  # [c, P, E, D, jl]
    Wp = Wt.reshape(n_cores, NP, 2 * E, C)
    Wb = Wp.reshape(n_cores, NBLK, 16, 2 * E, C).transpose(0, 1, 3, 2, 4)
    w_dev = np.ascontiguousarray(Wb).reshape(n_cores, NBLK, 128, 16 * C).astype(bf)

    # x: [B, P, E] -> block diag lhsT [128, NP*32], same on every core
    inpT = inp.transpose(1, 2, 0)          # [P, E, B]
    arr = inpT.reshape(NP, 2, E, B)
    x_dev = np.zeros((2, E, NP, 2, 16), np.float32)
    x_dev[0, :, :, 0, :] = arr[:, 0].transpose(1, 0, 2)
    x_dev[1, :, :, 1, :] = arr[:, 1].transpose(1, 0, 2)
    x_dev = x_dev.reshape(128, NP * 32).astype(bf)

    in_maps = []
    for c in range(n_cores):
        in_maps.append({"w": w_dev[c], "x": x_dev})
    return in_maps


def squash_np(x):
    s2 = np.sum(x * x, axis=-1, keepdims=True)
    return x * (s2 / (1.0 + s2)) / np.sqrt(s2)


def unshard(results):
    """Per-core 'out' [16, 4*C]: 4 stripe partials of s2 [16, (D, JL)].
    Sum stripes, squash, concatenate the disjoint j slices."""
    n_cores = len(results)
    JL = J // n_cores
    C = D * JL
    v = np.zeros((B, J, D), np.float32)
    for ci, r in enumerate(results):
        s = r["out"].astype(np.float64).reshape(16, 16, C).sum(axis=1)
        vj = squash_np(s.reshape(B, D, JL).transpose(0, 2, 1))
        v[:, ci * JL:(ci + 1) * JL, :] = vj
    return v


_CACHE = {}


def kernel(inp: np.ndarray, W: np.ndarray, b: np.ndarray) -> np.ndarray:
    n_cores = 8
    inp = np.asarray(inp, dtype=np.float32)
    W = np.asarray(W, dtype=np.float32)
    b = np.asarray(b, dtype=np.float32)

    if n_cores not in _CACHE:
        _CACHE[n_cores] = build_program(n_cores)
    nc = _CACHE[n_cores]

    in_maps = pack_inputs(inp, W, b, n_cores)
    res = run_bass_kernel_spmd(nc, in_maps, core_ids=list(range(n_cores)))
    return unshard(res.results)
